# revision 4
# baseline (speedup 1.0000x reference)
"""Trainium2 Bass kernel v2 for windowed multi-head attention (Pangu-style).

Math per window w (144 tokens, dim 192, 6 heads x 32):
  qkv = x @ w_qkv + b_qkv ; per head: S = (q*scale) @ k^T + bias[rel] ;
  masked softmax ; out = (softmax @ v) concat ; y = out @ w_out + b_out

Design (per core, 120 windows, batches of 3):
  - x transposed on HOST into xT_s [NB, 193, 432] f32 (row 192 = ones to
    fold biases); no device transposes/evacs for x.
  - qk generated fp32r weights-stationary, 4 chunks of 96 feature rows
    (q g0 | k g0 | q g1 | k g1), N=432; evac'd to fp16 SBUF.
  - scores fp16 block-diagonal: persistent zero-padded qzB [96, 864]
    (diag refreshed per window, 3 paired f16 copies); 4 matmuls
    [72, 432] into sct_jm [72, 1024]. All matmul operands at base
    partition 0 (HW rejects fp16 accumulation across partition offsets).
  - E = exp(scores^T) via 2 ACT ops -> [72, 1728] f16; ONE DVE multiply
    by M2 (host-precomputed exp(bias)*mask, fp16, compact layout).
  - attn@v token-major: lhsT = E slices [72m, 72n], rhs = [v_h | ones]
    [72m, 33] -> avp [72n, (jn)(h)(33)]; denominators land per-partition.
  - reciprocal on [72, (2)(6)(1)] (per-partition!), ONE fused
    normalize+evac tensor_tensor into attnTok [72, (2)(192)] f32r.
  - 4 PE transposes (f32r) -> attnT [97, 288] f16 (row 96 = ones via
    memset); outproj 4 matmuls N=192 fp16, b_out folded in wo1 row 96.
  - y written contiguously as [72, 3*384] f32 per batch; host fixes
    layout. DMAs batched per 3 windows (xT x2, m2 x1, y x1).
"""

import sys

sys.path.insert(0, "/opt/trn_rl_repo")

import numpy as np

DIM = 192
HEADS = 6
HD = 32
N = 144
NW = 960
NCORES = 8
WPC = NW // NCORES  # 120
NB3 = 3  # windows per batch

_NC_CACHE = {}


def _host_tensors(x, mask, rel_index, w_qkv, b_qkv, w_out, b_out, bias_table):
    f32, f16 = np.float32, np.float16
    scale = f32(1.0) / f32(np.sqrt(HD))
    w = np.array(w_qkv, f32, copy=True)
    b = np.array(b_qkv, f32, copy=True)
    w[:, :DIM] *= scale
    b[:DIM] *= scale

    # wqk [193, 384]: q (scaled) and k weights, bias as extra row.
    # Columns permuted into partition-aligned 96-row chunks: [q h0-2 |
    # k h0-2 | q h3-5 | k h3-5] so q_h and k_h share base partition offsets
    # in {0, 32, 64} (matmul operand base-partition constraint).
    wqk = np.concatenate([w[:, : 2 * DIM], b[None, : 2 * DIM]], axis=0)
    perm = np.concatenate([np.arange(0, 96), np.arange(192, 288),
                           np.arange(96, 192), np.arange(288, 384)])
    wqk = np.ascontiguousarray(wqk[:, perm])

    # wv [193, 256]: col 33h+d = v weights head h; col 33h+32 = ones-gen
    wv = np.zeros((DIM + 1, 256), f32)
    for h in range(HEADS):
        wv[:DIM, 33 * h : 33 * h + 32] = w[:, 2 * DIM + 32 * h : 2 * DIM + 32 * h + 32]
        wv[DIM, 33 * h : 33 * h + 32] = b[2 * DIM + 32 * h : 2 * DIM + 32 * h + 32]
        wv[DIM, 33 * h + 32] = 1.0

    # wo [193, 192] fp16: row 192 = b_out (multiplied by ones row of attnT)
    wo = np.concatenate([np.asarray(w_out, f32),
                         np.asarray(b_out, f32)[None, :]], axis=0).astype(f16)

    i72 = np.eye(72, dtype=f32)
    ones72 = np.ones((72, 1), f32)

    # xT_s [NB, 193, 432]: xT_s[b, f, 144wi+n] = x[3b+wi, n, f]; row 192 = 1
    xf = np.asarray(x, f32)
    nw = xf.shape[0]
    nb = nw // NB3
    xT = np.empty((nb, DIM + 1, NB3 * N), f32)
    xt = xf.reshape(nb, NB3, N, DIM).transpose(0, 3, 1, 2).reshape(nb, DIM, NB3 * N)
    xT[:, :DIM] = xt
    xT[:, DIM] = 1.0

    # M2 [w, mm, 864jm + 432g + 144hh + n] = expB[3g+hh, n, 72jm+mm]
    #   * mask[w, n, 72jm+mm]   (fp16)
    tab = np.asarray(bias_table, f32)
    ri = np.asarray(rel_index)
    expB = np.exp(tab[ri])                      # [n, m, H]
    # arrange expB -> [jm, g, hh, n, mm]
    eB = expB.reshape(N, 2, 72, HEADS).transpose(1, 3, 0, 2)  # [jm, H, n, mm]
    eB = eB.reshape(2, 2, 3, N, 72)                            # [jm, g, hh, n, mm]
    mk = np.asarray(mask, f32).reshape(nw, N, 2, 72)           # [w, n, jm, mm]
    m2 = (eB[None] * mk.transpose(0, 2, 1, 3)[:, :, None, None])  # [w,jm,g,hh,n,mm]
    M2 = np.ascontiguousarray(
        m2.transpose(0, 5, 1, 2, 3, 4).reshape(nw, 72, 2 * 2 * 3 * N)
    ).astype(f16)                                               # [w, mm, 1728]
    return wqk, wv, wo, i72, ones72, xT, M2


def build_nc(wpc=WPC, repeat=1):
    import concourse.bass as bass
    import concourse.mybir as mybir
    from concourse import bacc, tile
    from concourse.bass import broadcast_tensor_aps
    from contextlib import ExitStack

    f32 = mybir.dt.float32
    f32r = mybir.dt.float32r
    f16 = mybir.dt.float16
    Exp = mybir.ActivationFunctionType.Exp
    Copy = mybir.ActivationFunctionType.Copy
    MUL = mybir.AluOpType.mult

    assert wpc % NB3 == 0
    NB = wpc // NB3

    nc = bacc.Bacc("TRN2", target_bir_lowering=False, debug=False)
    xT_d = nc.declare_dram_parameter("xT_s", [NB, DIM + 1, NB3 * N], f32, isOutput=False)
    m2_d = nc.declare_dram_parameter("m2_s", [wpc // NB3, 72, NB3 * 1728], f16, isOutput=False)
    wqk_d = nc.declare_dram_parameter("wqk", [DIM + 1, 2 * DIM], f32, isOutput=False)
    wv_d = nc.declare_dram_parameter("wv", [DIM + 1, 256], f32, isOutput=False)
    wo_d = nc.declare_dram_parameter("wo", [DIM + 1, DIM], f16, isOutput=False)
    i72_d = nc.declare_dram_parameter("i72", [72, 72], f32, isOutput=False)
    on72_d = nc.declare_dram_parameter("ones72", [72, 1], f32, isOutput=False)
    y_d = nc.declare_dram_parameter("y_s", [wpc // NB3, 72, NB3 * 2 * DIM], f32, isOutput=True)

    def r(ap):
        return ap.bitcast(f32r)

    with ExitStack() as ctx:
        tc = ctx.enter_context(tile.TileContext(nc))
        cpool = ctx.enter_context(tc.tile_pool(name="const", bufs=1))
        sb = ctx.enter_context(tc.tile_pool(name="sb", bufs=2))
        pp = ctx.enter_context(tc.tile_pool(name="pp", bufs=1, space="PSUM"))

        # ---- constants ----
        wqk0 = cpool.tile([96, 2 * DIM], f32)
        wqk1 = cpool.tile([97, 2 * DIM], f32)
        nc.sync.dma_start(out=r(wqk0[:]), in_=r(wqk_d[0:96, :]))
        nc.sync.dma_start(out=r(wqk1[:]), in_=r(wqk_d[96:193, :]))
        wv0 = cpool.tile([96, 256], f32)
        wv1 = cpool.tile([97, 256], f32)
        nc.sync.dma_start(out=r(wv0[:]), in_=r(wv_d[0:96, :]))
        nc.sync.dma_start(out=r(wv1[:]), in_=r(wv_d[96:193, :]))
        wo0 = cpool.tile([96, DIM], f16)
        wo1 = cpool.tile([97, DIM], f16)
        nc.sync.dma_start(out=wo0[:], in_=wo_d[0:96, :])
        nc.sync.dma_start(out=wo1[:], in_=wo_d[96:193, :])
        i72 = cpool.tile([72, 72], f32)
        nc.sync.dma_start(out=r(i72[:]), in_=r(i72_d[:]))
        on72 = cpool.tile([72, 1], f32)
        nc.sync.dma_start(out=r(on72[:]), in_=r(on72_d[:]))

        # persistent zero-padded block-diagonal q tiles [96, (2g)(3hh*144)],
        # one per window parity; off-diagonal zeros persist, diagonal blocks
        # refreshed per window.
        qzB = [cpool.tile([96, 864], f16, name=f"qzB{p}") for p in range(2)]
        for p in range(2):
            nc.vector.memset(qzB[p][:], 0.0)

        if repeat > 1:
            ctx.enter_context(tc.For_i(0, repeat, 1))

        for b in range(NB):
            # ---- x^T batch tiles (host-transposed) ----
            xt0 = sb.tile([96, NB3 * N], f32, tag="xt0")
            xt1 = sb.tile([97, NB3 * N], f32, tag="xt1")
            nc.sync.dma_start(out=r(xt0[:]), in_=r(xT_d[b, 0:96, :]))
            nc.sync.dma_start(out=r(xt1[:]), in_=r(xT_d[b, 96:193, :]))

            # ---- qk generation: 4 chunks of 96 feature rows ----
            # chunk0 = q g0, chunk1 = k g0, chunk2 = q g1, chunk3 = k g1
            qq = sb.tile([96, 2, NB3 * N], f16, tag="qq")
            ksb = [sb.tile([96, NB3 * N], f16, tag=f"ksb{g}", name=f"ksb{g}_{b}")
                   for g in range(2)]
            dsts = [qq[:, 0, :], ksb[0][:], qq[:, 1, :], ksb[1][:]]
            for c in range(4):
                qkp = pp.tile([96, NB3 * N], f32, tag="qkvp", bufs=2)
                nc.tensor.matmul(qkp[:], r(wqk0[:, 96 * c : 96 * c + 96]),
                                 r(xt0[:]), start=True, stop=False)
                nc.tensor.matmul(qkp[:], r(wqk1[:, 96 * c : 96 * c + 96]),
                                 r(xt1[:]), start=False, stop=True)
                if c % 2 == 0:
                    nc.vector.tensor_copy(dsts[c], qkp[:])
                else:
                    nc.scalar.activation(dsts[c], qkp[:], Copy)

            m2b = sb.tile([72, NB3 * 1728], f16, tag="m2", bufs=3)
            nc.scalar.dma_start(out=m2b[:], in_=m2_d[b])
            yb = sb.tile([72, NB3 * 2 * DIM], f32, tag="yb", bufs=2)

            for wi in range(NB3):
                w = NB3 * b + wi
                tw = slice(N * wi, N * wi + N)

                def s72(jm):
                    return slice(N * wi + 72 * jm, N * wi + 72 * jm + 72)

                m2t = m2b[:, 1728 * wi : 1728 * wi + 1728]

                # ---- v generation: [72, (2jm)(256)], fp32r N=256 ----
                vp = pp.tile([72, 512], f32, tag="qkvp", bufs=2)
                for jm in range(2):
                    nc.tensor.matmul(vp[:, 256 * jm : 256 * jm + 256],
                                     r(xt0[:, s72(jm)]), r(wv0[:]),
                                     start=True, stop=False)
                    nc.tensor.matmul(vp[:, 256 * jm : 256 * jm + 256],
                                     r(xt1[:, s72(jm)]), r(wv1[:]),
                                     start=False, stop=True)
                vt = sb.tile([72, 2, 198], f16, tag="vt", bufs=3)
                nc.scalar.activation(
                    vt[:], vp[:].rearrange("p (j c) -> p j c", j=2)[:, :, 0:198],
                    Copy)

                # ---- qz diagonal refresh: per hh, both g at once ----
                par = w % 2
                for hh in range(3):
                    dst = qzB[par][32 * hh : 32 * hh + 32, :].rearrange(
                        "p (g c) -> p g c", g=2)[:, :, 144 * hh : 144 * hh + 144]
                    srcv = qq[32 * hh : 32 * hh + 32, :, tw]
                    nc.vector.tensor_copy(dst, srcv)

                # ---- scores (fp16, block-diag 3 heads/matmul) ----
                E = sb.tile([72, 1728], f16, tag="E", bufs=3)
                for jm in range(2):
                    sct = pp.tile([72, 1024], f32, tag="sct", bufs=2)
                    for g in range(2):
                        nc.tensor.matmul(
                            sct[:, 512 * g : 512 * g + 432],
                            ksb[g][:, s72(jm)],
                            qzB[par][:, 432 * g : 432 * g + 432],
                            start=True, stop=True)
                    # E[:, 864jm + 432g + n'] = exp(sct[:, 512g + n'])
                    nc.scalar.activation(
                        E[:, 864 * jm : 864 * jm + 864],
                        sct[:].rearrange("p (g c) -> p g c", g=2)[:, :, 0:432],
                        Exp)
                # ---- mask+bias multiply ----
                nc.vector.tensor_tensor(E[:], E[:], m2t[:], MUL)

                # ---- attn @ v, token-major: avp [72n', (2jn)(6h)(33)] ----
                avp = pp.tile([72, 2, 6, 33], f32, tag="avp", bufs=1)
                for jn in range(2):
                    for h in range(HEADS):
                        g, hh = divmod(h, 3)
                        for jm in range(2):
                            col = 864 * jm + 432 * g + 144 * hh + 72 * jn
                            nc.tensor.matmul(
                                avp[:, jn, h, :],
                                E[:, col : col + 72],
                                vt[:, jm, 33 * h : 33 * h + 33],
                                start=(jm == 0), stop=(jm == 1))

                # ---- softmax normalize (per-partition!) + evac ----
                rden = sb.tile([72, 2, 6, 1], f32, tag="rden", bufs=3)
                with nc.allow_low_precision("softmax denominators"):
                    nc.vector.reciprocal(rden[:], avp[:, :, :, 32:33])
                attnTok = sb.tile([72, 2, 192], f32, tag="atok", bufs=3)
                a_out = attnTok[:].rearrange("p j (h d) -> p j h d", h=6)
                a_in0, a_in1 = broadcast_tensor_aps(avp[:, :, :, 0:32], rden[:])
                nc.vector.tensor_tensor(r(a_out), a_in0, a_in1, MUL)

                # ---- transpose to feature-major [96, (2fh)(2jn)(72)] ----
                Tp = pp.tile([96, 288], f32, tag="Tppf", bufs=1)
                for jn in range(2):
                    nc.tensor.transpose(r(Tp[0:96, 72 * jn : 72 * jn + 72]),
                                        r(attnTok[:, jn, 0:96]), r(i72[:]))
                    nc.tensor.transpose(r(Tp[0:96, 144 + 72 * jn : 216 + 72 * jn]),
                                        r(attnTok[:, jn, 96:192]), r(i72[:]))
                attnT = sb.tile([97, 288], f16, tag="attnT", bufs=3)
                nc.vector.memset(attnT[96:97, 144:288], 1.0)
                nc.vector.tensor_copy(attnT[0:96, :], Tp[:])

                # ---- output projection (fp16, b_out in wo1 row 96) ----
                pf = pp.tile([72, 2 * DIM], f32, tag="Tppf", bufs=1)
                for jn in range(2):
                    nc.tensor.matmul(pf[:, DIM * jn : DIM * jn + DIM],
                                     attnT[0:96, 72 * jn : 72 * jn + 72],
                                     wo0[:], start=True, stop=False)
                    nc.tensor.matmul(pf[:, DIM * jn : DIM * jn + DIM],
                                     attnT[0:97, 144 + 72 * jn : 216 + 72 * jn],
                                     wo1[:], start=False, stop=True)
                nc.scalar.activation(
                    yb[:, 384 * wi : 384 * wi + 384], pf[:], Copy)
            nc.sync.dma_start(out=y_d[b], in_=yb[:])

    nc.compile()
    return nc


def make_in_maps(inputs):
    wqk, wv, wo, i72, ones72, xT, M2 = _host_tensors(**inputs)
    # M2 [w, 72, 1728] -> per-batch [NB, 72, (3 wi)(1728)]
    nb_all = M2.shape[0] // NB3
    M2b = np.ascontiguousarray(
        M2.reshape(nb_all, NB3, 72, 1728).transpose(0, 2, 1, 3).reshape(
            nb_all, 72, NB3 * 1728))
    in_maps = []
    for c in range(NCORES):
        slb = slice(WPC // NB3 * c, WPC // NB3 * (c + 1))
        in_maps.append({
            "xT_s": np.ascontiguousarray(xT[slb]),
            "m2_s": np.ascontiguousarray(M2b[slb]),
            "wqk": wqk, "wv": wv, "wo": wo, "i72": i72,
            "ones72": ones72,
        })
    return in_maps


def kernel(x, mask, rel_index, w_qkv, b_qkv, w_out, b_out, bias_table):
    if WPC not in _NC_CACHE:
        _NC_CACHE[WPC] = build_nc(WPC)
    nc = _NC_CACHE[WPC]
    in_maps = make_in_maps(dict(
        x=x, mask=mask, rel_index=rel_index, w_qkv=w_qkv, b_qkv=b_qkv,
        w_out=w_out, b_out=b_out, bias_table=bias_table))

    from concourse.bass_utils import run_bass_kernel_spmd
    res = run_bass_kernel_spmd(nc, in_maps, list(range(NCORES)))
    global LAST_EXEC_NS, LAST_RESULTS
    LAST_EXEC_NS = res.exec_time_ns
    LAST_RESULTS = res
    # y_s [NB, 72, (3 wi)(2 jn)(192)] -> y [wpc, 144, 192]
    outs = []
    for c in range(NCORES):
        ys = res.results[c]["y_s"].reshape(WPC // NB3, 72, NB3, 2, DIM)
        outs.append(ys.transpose(0, 2, 3, 1, 4).reshape(WPC, N, DIM))
    return np.concatenate(outs, axis=0).astype(np.float32)
